# revision 7
# baseline (speedup 1.0000x reference)
"""Trainium2 Bass kernel for nn_L2MLoRAqkv (MoE-routed LoRA QKV projection).

Math (per batch b, expert i = idx[b,0]):
    qkv = x @ W.T + bias
    qkv[:, :D]  += (x @ A_q[i]) @ B_q[i] * SCALE
    qkv[:, -D:] += (x @ A_v[i]) @ B_v[i] * SCALE

Strategy: data-parallel over the batch dim (1 batch per NeuronCore, 8 cores).
On the host we gather each batch's expert and fold the rank-8 LoRA update
into the (transposed) projection weight in float64:
    W_eff[b] = W.T; W_eff[:, :D] += A_q[i] @ B_q[i]; W_eff[:, -D:] += A_v[i] @ B_v[i]
so the device kernel is a single dense GEMM per core:
    Y[4096, 3072] = X[4096, 1024] @ W_eff[1024, 3072] + bias

The GEMM runs on the PE in fp8(e4m3) DoubleRow mode (0.5 cycles/row, two
k-tiles per instruction = 4x bf16 row throughput).  To hit the 2e-2 accuracy
gate both operands are split hi/lo against a shared power-of-2 scale:
    x*sX  = xhi8 + xlo8 + eps_x      (eps ~ 7e-4 relative)
    W*sW  = whi8 + wlo8 + eps_w
    y ~= [(xhi8+xlo8) @ whi8 + xhi8 @ wlo8] / (sX*sW) + bias
All three partial GEMMs share one PSUM accumulation chain (same scale), so a
(t, n) output tile is 12 DoubleRow matmuls (24 k-tile products).  Measured
end-to-end scheme error vs the f64 reference is ~1e-3.

X is pre-transposed on the host ([D, T], K-major) so both matmul operands
load with K on SBUF partitions via contiguous DMAs.  Output is stored fp16
and upcast on the host.
"""

import os
import sys

import numpy as np

for _p in ("/opt/trn_rl_repo",):
    if _p not in sys.path and os.path.isdir(_p):
        sys.path.insert(0, _p)

B = 8          # batches == cores
T = 4096       # tokens per batch
D = 1024       # model dim (contraction K)
N3 = 3072      # qkv output dim
P = 128        # SBUF partitions
NT = 512       # n-tile (one fp32 PSUM bank)
CHUNK = 1024   # token chunk streamed per DMA group (1 KiB fp8 rows)
KT = D // P        # 8 k-tiles
KP = KT // 2       # 4 k-pairs (DoubleRow takes 2 k-tiles per matmul)
NN = N3 // NT      # 6 n-tiles
TT = CHUNK // P    # 8 token sub-tiles per chunk
NCH = T // CHUNK   # 4 chunks
SCALE = 8.0 / 8.0

SX = 32.0      # x fp8 scale (abs(x*SX) <= ~180 < 240 = e4m3 max)
SW = 1024.0    # W fp8 scale (abs(W*SW) <= ~120)
INV = 1.0 / (SX * SW)

WARMUP = 96    # dummy matmuls to ramp the PE p-state while DMAs land

_NC_CACHE = {}


def _build(tokens=T):
    import concourse.tile as tile
    from concourse import bacc, mybir

    f8 = mybir.dt.float8e4
    f16 = mybir.dt.float16
    f32 = mybir.dt.float32
    DR = mybir.MatmulPerfMode.DoubleRow

    nc = bacc.Bacc(
        "TRN2",
        target_bir_lowering=False,
        debug=False,
        enable_asserts=False,
        num_devices=B,
    )
    xhi_d = nc.dram_tensor("xhi", [D, tokens], f8, kind="ExternalInput").ap()
    xlo_d = nc.dram_tensor("xlo", [D, tokens], f8, kind="ExternalInput").ap()
    whi_d = nc.dram_tensor("whi", [D, N3], f8, kind="ExternalInput").ap()
    wlo_d = nc.dram_tensor("wlo", [D, N3], f8, kind="ExternalInput").ap()
    biasr = nc.dram_tensor("biasr", [P, N3], f16, kind="ExternalInput").ap()
    y = nc.dram_tensor("y", [tokens, N3], f16, kind="ExternalOutput").ap()

    ld_ctr = [0]
    st_ctr = [0]

    with tile.TileContext(nc) as tc:
        with tc.tile_pool(name="const", bufs=1) as const_pool, \
             tc.tile_pool(name="xin", bufs=3) as xin_pool, \
             tc.tile_pool(name="tmp", bufs=4) as tmp_pool, \
             tc.tile_pool(name="outp", bufs=4) as out_pool, \
             tc.tile_pool(name="ps", bufs=8, space="PSUM") as psum_pool:

            def ld_eng():
                ld_ctr[0] += 1
                return nc.scalar if ld_ctr[0] % 2 else nc.sync

            def st_eng():
                st_ctr[0] += 1
                return nc.scalar if st_ctr[0] % 2 else nc.sync

            # --- PE warmup: ramp the p-state while startup DMAs land. ---
            wz = const_pool.tile([P, 2, NT], f8)
            nc.vector.memset(wz[:], 0.0)
            wps = psum_pool.tile([P, NT], f32, tag="ps", name="ps")
            for _ in range(WARMUP):
                nc.tensor.matmul(
                    wps[:],
                    lhsT=wz[:, :, 0:P],
                    rhs=wz[:],
                    start=True,
                    stop=True,
                    perf_mode=DR,
                )

            # --- streaming x chunks (hi+lo classes) ---
            def load_chunk(c):
                xh = xin_pool.tile([P, KT, CHUNK], f8, tag="xh", name="xh")
                xl = xin_pool.tile([P, KT, CHUNK], f8, tag="xl", name="xl")
                for cls_t, cls_d in ((xh, xhi_d), (xl, xlo_d)):
                    for k in range(KT):
                        ld_eng().dma_start(
                            cls_t[:, k, :],
                            cls_d[k * P : (k + 1) * P, c * CHUNK : (c + 1) * CHUNK],
                        )
                return xh, xl

            # Critical startup order: xhi c0, W n0 (hi+lo), xlo c0, then the
            # rest of W n-major so matmul groups unblock in arrival order.
            xh0 = xin_pool.tile([P, KT, CHUNK], f8, tag="xh", name="xh")
            for k in range(KT):
                ld_eng().dma_start(xh0[:, k, :], xhi_d[k * P : (k + 1) * P, 0:CHUNK])

            whi_sb = const_pool.tile([P, KT, N3], f8)
            wlo_sb = const_pool.tile([P, KT, N3], f8)

            def load_w_slice(n):
                for w_sb, w_d in ((whi_sb, whi_d), (wlo_sb, wlo_d)):
                    for k in range(KT):
                        ld_eng().dma_start(
                            w_sb[:, k, n * NT : (n + 1) * NT],
                            w_d[k * P : (k + 1) * P, n * NT : (n + 1) * NT],
                        )

            load_w_slice(0)

            xl0 = xin_pool.tile([P, KT, CHUNK], f8, tag="xl", name="xl")
            for k in range(KT):
                ld_eng().dma_start(xl0[:, k, :], xlo_d[k * P : (k + 1) * P, 0:CHUNK])

            bias_sb = const_pool.tile([P, N3], f16)
            nc.gpsimd.dma_start(bias_sb[:], biasr[:])

            for n in range(1, NN):
                load_w_slice(n)

            def drain(ps, c, t, n):
                tm = tmp_pool.tile([P, NT], f16, tag="tm", name="tm")
                nc.scalar.activation(
                    tm[:], ps[:], mybir.ActivationFunctionType.Copy, scale=INV
                )
                ob = out_pool.tile([P, NT], f16, tag="ob", name="ob")
                nc.vector.tensor_add(ob[:], tm[:], bias_sb[:, n * NT : (n + 1) * NT])
                # Split the store across both HWDGE rings (parallel queues).
                row = c * CHUNK + t * P
                h = P // 2
                st_eng().dma_start(
                    y[row : row + h, n * NT : (n + 1) * NT], ob[0:h, :]
                )
                st_eng().dma_start(
                    y[row + h : row + P, n * NT : (n + 1) * NT], ob[h:P, :]
                )

            # The 12 DoubleRow matmuls of one (t, n) accumulation chain.
            # Order: (xhi@whi, xhi@wlo) per k-pair — consecutive instructions
            # share the stationary lhsT — then the 4 xlo@whi.
            def hi_steps(xh, t, n, ps, start):
                ts_ = slice(t * P, (t + 1) * P)
                ns_ = slice(n * NT, (n + 1) * NT)
                for kp in range(KP):
                    ks = slice(2 * kp, 2 * kp + 2)
                    nc.tensor.matmul(
                        ps[:], lhsT=xh[:, ks, ts_], rhs=whi_sb[:, ks, ns_],
                        start=(start and kp == 0), stop=False, perf_mode=DR,
                    )
                    nc.tensor.matmul(
                        ps[:], lhsT=xh[:, ks, ts_], rhs=wlo_sb[:, ks, ns_],
                        start=False, stop=False, perf_mode=DR,
                    )

            def lo_steps(xl, t, n, ps):
                ts_ = slice(t * P, (t + 1) * P)
                ns_ = slice(n * NT, (n + 1) * NT)
                for kp in range(KP):
                    ks = slice(2 * kp, 2 * kp + 2)
                    nc.tensor.matmul(
                        ps[:], lhsT=xl[:, ks, ts_], rhs=whi_sb[:, ks, ns_],
                        start=False, stop=(kp == KP - 1), perf_mode=DR,
                    )

            # --- head chunk c0: n-outer so groups follow W arrival order.
            # n0 is split into a hi pass and a lo pass so the PE can start on
            # xhi/whi/wlo before xlo lands.
            pss0 = []
            for t in range(TT):
                ps = psum_pool.tile([P, NT], f32, tag="ps", name="ps")
                hi_steps(xh0, t, 0, ps, start=True)
                pss0.append(ps)
            for t in range(TT):
                lo_steps(xl0, t, 0, pss0[t])
                drain(pss0[t], 0, t, 0)
            for n in range(1, NN):
                for t in range(TT):
                    ps = psum_pool.tile([P, NT], f32, tag="ps", name="ps")
                    hi_steps(xh0, t, n, ps, start=True)
                    lo_steps(xl0, t, n, ps)
                    drain(ps, 0, t, n)

            # --- steady chunks: n-inner (lhsT reused across the 6 n-tiles).
            nxt = load_chunk(1) if NCH > 1 else None
            for c in range(1, NCH):
                xh, xl = nxt
                nxt = load_chunk(c + 1) if c + 1 < NCH else None
                for t in range(TT):
                    ts_ = slice(t * P, (t + 1) * P)
                    pss = [
                        psum_pool.tile([P, NT], f32, tag="ps", name="ps")
                        for _ in range(NN)
                    ]
                    for kp in range(KP):
                        ks = slice(2 * kp, 2 * kp + 2)
                        for w_sb in (whi_sb, wlo_sb):
                            for n in range(NN):
                                nc.tensor.matmul(
                                    pss[n][:],
                                    lhsT=xh[:, ks, ts_],
                                    rhs=w_sb[:, ks, slice(n * NT, (n + 1) * NT)],
                                    start=(kp == 0 and w_sb is whi_sb),
                                    stop=False,
                                    perf_mode=DR,
                                )
                    for kp in range(KP):
                        ks = slice(2 * kp, 2 * kp + 2)
                        for n in range(NN):
                            nc.tensor.matmul(
                                pss[n][:],
                                lhsT=xl[:, ks, ts_],
                                rhs=whi_sb[:, ks, slice(n * NT, (n + 1) * NT)],
                                start=False,
                                stop=(kp == KP - 1),
                                perf_mode=DR,
                            )
                    for n in range(NN):
                        drain(pss[n], c, t, n)
    nc.compile()
    return nc


def _get_nc(tokens=T):
    key = tokens
    if key not in _NC_CACHE:
        _NC_CACHE[key] = _build(tokens)
    return _NC_CACHE[key]


def _quant_hilo(a32, s, fp8):
    hi8 = (a32 * s).astype(fp8)
    lo8 = (a32 * s - hi8.astype(np.float32)).astype(fp8)
    return hi8, lo8


def _prep_in_maps(inputs):
    from concourse import mybir

    fp8 = mybir.dt.np(mybir.dt.float8e4)

    x = np.asarray(inputs["x"], dtype=np.float32)
    weight = np.asarray(inputs["weight"], dtype=np.float32)
    bias = np.asarray(inputs["bias"], dtype=np.float32)
    aq = np.asarray(inputs["A_q_pool"], dtype=np.float32)
    bq = np.asarray(inputs["B_q_pool"], dtype=np.float32)
    av = np.asarray(inputs["A_v_pool"], dtype=np.float32)
    bv = np.asarray(inputs["B_v_pool"], dtype=np.float32)
    idx = np.asarray(inputs["idx"]).reshape(B, -1)[:, 0].astype(np.int64)

    wt64 = weight.T.astype(np.float64)  # [D, N3]
    biasr = np.ascontiguousarray(
        np.broadcast_to(bias.astype(np.float16), (P, N3))
    )
    xts = x.transpose(0, 2, 1)  # [B, D, T] strided view

    in_maps = []
    for b in range(B):
        i = int(idx[b])
        weff = wt64.copy()
        weff[:, :D] += SCALE * (aq[i].astype(np.float64) @ bq[i].astype(np.float64))
        weff[:, N3 - D:] += SCALE * (av[i].astype(np.float64) @ bv[i].astype(np.float64))
        whi8, wlo8 = _quant_hilo(weff.astype(np.float32), SW, fp8)
        xhi8, xlo8 = _quant_hilo(np.ascontiguousarray(xts[b]), SX, fp8)
        in_maps.append({
            "xhi": xhi8, "xlo": xlo8,
            "whi": whi8, "wlo": wlo8,
            "biasr": biasr,
        })
    return in_maps


def _run(in_maps, trace=False, **kwargs):
    from concourse.bass_utils import run_bass_kernel_spmd

    nc = _get_nc()
    return run_bass_kernel_spmd(
        nc, in_maps, core_ids=list(range(B)), trace=trace, **kwargs
    )


def kernel(**inputs):
    res = _run(_prep_in_maps(inputs), trace=False)
    return np.stack([r["y"].astype(np.float32) for r in res.results], axis=0)


# revision 8
# speedup vs baseline: 1.5014x; 1.5014x over previous
"""Trainium2 Bass kernel for nn_L2MLoRAqkv (MoE-routed LoRA QKV projection).

Math (per batch b, expert i = idx[b,0]):
    qkv = x @ W.T + bias
    qkv[:, :D]  += (x @ A_q[i]) @ B_q[i] * SCALE
    qkv[:, -D:] += (x @ A_v[i]) @ B_v[i] * SCALE

Strategy: data-parallel over the batch dim (1 batch per NeuronCore, 8 cores).
On the host we gather each batch's expert and fold the rank-8 LoRA update
into the (transposed) projection weight in float64:
    W_eff[b] = W.T; W_eff[:, :D] += A_q[i] @ B_q[i]; W_eff[:, -D:] += A_v[i] @ B_v[i]
so the device kernel is a single dense GEMM per core:
    Y[4096, 3072] = X[4096, 1024] @ W_eff[1024, 3072] + bias

Everything runs in fp16 (PE: 1 cycle/row, same as f32r, at half the DMA
bytes; end-to-end error ~3e-4 vs the 2e-2 gate).  X is pre-transposed on the
host ([D, T], K-major) so both matmul operands load with K on SBUF
partitions via contiguous DMAs.  The output is stored fp16 and upcast on the
host.

Pipeline details:
- ~8 us of fixed NEFF startup happens before any engine slice; a block of
  dummy matmuls on zeroed SBUF ramps the PE p-state (0.65 -> 2.4 GHz takes
  ~3 us of continuous work) while the first real DMAs land.
- The first x chunk and the first W n-slice are DMA'd as [64, .] partition
  halves so the critical tiles spread across more of the 16 DMA queues
  (a [128, 512] fp16 tile is 128 descriptors on ONE queue ~ 6 us).
- Output stores split into two partition halves across both HWDGE rings.
"""

import os
import sys

import numpy as np

for _p in ("/opt/trn_rl_repo",):
    if _p not in sys.path and os.path.isdir(_p):
        sys.path.insert(0, _p)

B = 8          # batches == cores
T = 4096       # tokens per batch
D = 1024       # model dim (contraction K)
N3 = 3072      # qkv output dim
P = 128        # SBUF partitions
NT = 512       # n-tile (one fp32 PSUM bank)
CHUNK = 512    # token chunk streamed per DMA group
KT = D // P        # 8 k-tiles
NN = N3 // NT      # 6 n-tiles
TT = CHUNK // P    # 4 token sub-tiles per chunk
SCALE = 8.0 / 8.0

WARMUP = 40    # dummy matmuls to ramp the PE p-state while DMAs land

_NC_CACHE = {}


def _build(tokens=T):
    import concourse.tile as tile
    from concourse import bacc, mybir

    nchunk = tokens // CHUNK
    f16 = mybir.dt.float16
    f32 = mybir.dt.float32

    nc = bacc.Bacc(
        "TRN2",
        target_bir_lowering=False,
        debug=False,
        enable_asserts=False,
        num_devices=B,
    )
    xt = nc.dram_tensor("xt", [D, tokens], f16, kind="ExternalInput").ap()
    weff = nc.dram_tensor("weff", [D, N3], f16, kind="ExternalInput").ap()
    biasr = nc.dram_tensor("biasr", [P, N3], f16, kind="ExternalInput").ap()
    y = nc.dram_tensor("y", [tokens, N3], f16, kind="ExternalOutput").ap()

    ld_ctr = [0]
    st_ctr = [0]

    with tile.TileContext(nc) as tc:
        with tc.tile_pool(name="const", bufs=1) as const_pool, \
             tc.tile_pool(name="xin", bufs=3) as xin_pool, \
             tc.tile_pool(name="outp", bufs=4) as out_pool, \
             tc.tile_pool(name="ps", bufs=8, space="PSUM") as psum_pool:

            def ld_eng():
                ld_ctr[0] += 1
                return nc.scalar if ld_ctr[0] % 2 else nc.sync

            def st_eng():
                st_ctr[0] += 1
                return nc.scalar if st_ctr[0] % 2 else nc.sync

            # --- PE warmup: ramp the p-state while startup DMAs land. ---
            wz = const_pool.tile([P, NT], f16)
            nc.vector.memset(wz[:], 0.0)
            wps = psum_pool.tile([P, NT], f32, tag="ps", name="ps")
            for _ in range(WARMUP):
                nc.tensor.matmul(
                    wps[:], lhsT=wz[:, 0:P], rhs=wz[:], start=True, stop=True
                )

            def load_chunk(c, split=False):
                # X.T chunk: 8 k-tiles of [128, CHUNK] side by side.
                xc = xin_pool.tile([P, KT, CHUNK], f16, tag="xc", name="xc")
                for k in range(KT):
                    src = xt[k * P : (k + 1) * P, c * CHUNK : (c + 1) * CHUNK]
                    if split:
                        h = P // 2
                        ld_eng().dma_start(xc[0:h, k, :], src[0:h, :])
                        ld_eng().dma_start(xc[h:P, k, :], src[h:P, :])
                    else:
                        ld_eng().dma_start(xc[:, k, :], src)
                return xc

            # Chunk 0 of X first (partition-split for queue parallelism).
            head = 1
            xcs_head = [load_chunk(0, split=True)]

            # W_eff resident in SBUF: [128, 8, 3072].  n-slice-major so the
            # first matmul groups unblock early; n0 partition-split.
            w_sb = const_pool.tile([P, KT, N3], f16)
            for n in range(NN):
                for k in range(KT):
                    src = weff[k * P : (k + 1) * P, n * NT : (n + 1) * NT]
                    if n == 0:
                        h = P // 2
                        ld_eng().dma_start(w_sb[0:h, k, n * NT : (n + 1) * NT], src[0:h, :])
                        ld_eng().dma_start(w_sb[h:P, k, n * NT : (n + 1) * NT], src[h:P, :])
                    else:
                        ld_eng().dma_start(w_sb[:, k, n * NT : (n + 1) * NT], src)
                if n == 0:
                    bias_sb = const_pool.tile([P, N3], f16)
                    nc.gpsimd.dma_start(bias_sb[:], biasr[:])

            def drain(ps, c, t, n):
                ob = out_pool.tile([P, NT], f16, tag="ob", name="ob")
                nc.vector.tensor_add(ob[:], ps[:], bias_sb[:, n * NT : (n + 1) * NT])
                # Split the store across both HWDGE rings (parallel queues).
                row = c * CHUNK + t * P
                h = P // 2
                st_eng().dma_start(y[row : row + h, n * NT : (n + 1) * NT], ob[0:h, :])
                st_eng().dma_start(y[row + h : row + P, n * NT : (n + 1) * NT], ob[h:P, :])

            def do_group(xc, c, t, n):
                ps = psum_pool.tile([P, NT], f32, tag="ps", name="ps")
                for k in range(KT):
                    nc.tensor.matmul(
                        ps[:],
                        lhsT=xc[:, k, t * P : (t + 1) * P],
                        rhs=w_sb[:, k, n * NT : (n + 1) * NT],
                        start=(k == 0),
                        stop=(k == KT - 1),
                    )
                drain(ps, c, t, n)

            # Head chunk n-outer, so matmul groups unblock in weff
            # DMA-arrival order and never outrun the loads.
            for n in range(NN):
                for c in range(head):
                    for t in range(TT):
                        do_group(xcs_head[c], c, t, n)

            # Remaining chunks: weff fully resident.
            for c in range(head, nchunk):
                xc = load_chunk(c)
                for t in range(TT):
                    for n in range(NN):
                        do_group(xc, c, t, n)
    nc.compile()
    return nc


def _get_nc(tokens=T):
    if tokens not in _NC_CACHE:
        _NC_CACHE[tokens] = _build(tokens)
    return _NC_CACHE[tokens]


def _prep_in_maps(inputs):
    x = np.asarray(inputs["x"], dtype=np.float32)
    weight = np.asarray(inputs["weight"], dtype=np.float32)
    bias = np.asarray(inputs["bias"], dtype=np.float32)
    aq = np.asarray(inputs["A_q_pool"], dtype=np.float32)
    bq = np.asarray(inputs["B_q_pool"], dtype=np.float32)
    av = np.asarray(inputs["A_v_pool"], dtype=np.float32)
    bv = np.asarray(inputs["B_v_pool"], dtype=np.float32)
    idx = np.asarray(inputs["idx"]).reshape(B, -1)[:, 0].astype(np.int64)

    wt64 = weight.T.astype(np.float64)  # [D, N3]
    biasr = np.ascontiguousarray(np.broadcast_to(bias.astype(np.float16), (P, N3)))
    xts = x.transpose(0, 2, 1)  # [B, D, T] strided view

    in_maps = []
    for b in range(B):
        i = int(idx[b])
        weff = wt64.copy()
        weff[:, :D] += SCALE * (aq[i].astype(np.float64) @ bq[i].astype(np.float64))
        weff[:, N3 - D:] += SCALE * (av[i].astype(np.float64) @ bv[i].astype(np.float64))
        in_maps.append({
            "xt": np.ascontiguousarray(xts[b]).astype(np.float16),
            "weff": weff.astype(np.float16),
            "biasr": biasr,
        })
    return in_maps


def _run(in_maps, trace=False, **kwargs):
    from concourse.bass_utils import run_bass_kernel_spmd

    nc = _get_nc()
    return run_bass_kernel_spmd(
        nc, in_maps, core_ids=list(range(B)), trace=trace, **kwargs
    )


def kernel(**inputs):
    res = _run(_prep_in_maps(inputs), trace=False)
    return np.stack([r["y"].astype(np.float32) for r in res.results], axis=0)
